# revision 2
# baseline (speedup 1.0000x reference)
import sys
for _p in ("/opt/trn_rl_repo",):
    if _p not in sys.path:
        sys.path.insert(0, _p)
"""Causal self-attention TRN2 kernel, 8-core SPMD — fused pipeline v2.

Sharding: core c handles batch b = c//2 and head-half h = c%2 (8 of 16 heads,
i.e. 512 of 1024 q/k/v channels). Host sums the two output-projection partials
per batch and concatenates batches.

v2 layout (vs v1 phase-separated):
  - x resident in SBUF whole kernel; V computed first (dense PE warm-up for
    the HAM clock gate), then per channel-group g (= head pair): q/k projected
    straight into SBUF (no DRAM bounce), attention for g immediately after.
  - Projection matmuls for group g+1 are interleaved into group g's attention
    j-loop so the PE has dense work while ACT (exp) paces the softmax.
  - hh processed serially (not tile-position-packed) to fit PSUM in 8 banks:
    S double-buffer 2x[128,1024] (4) + y accum 2x[65,512] (2) + shared
    accumulation ring 2x[128,512] (2).
  - Softmax denominators via ones-column in V (y row 64); per-g batched
    reciprocal bounced through DRAM (cross-partition gather), normalize
    overlapped under the next group's attention.
"""

import numpy as np
import concourse.bass as bass
import concourse.mybir as mybir
import concourse.tile as tile
from concourse import bacc

P = 128
T = 2048          # tokens per batch
F = 1024          # model dim (projection contraction)
CH = 512          # per-core q/k/v channels (8 heads x 64)
NH = 8            # heads per core
D = 64
NKO = F // P      # 8
NM = CH // P      # 4 channel tiles (= head pairs)
GSZ = 512         # tq group size
NG = T // GSZ     # 4
NTT = T // P      # 16
OUTC = 1024
NCHAIN = NM * NG * 2   # 32 softmax-denominator chains

f32 = mybir.dt.float32
f32r = mybir.dt.float32r
bf16 = mybir.dt.bfloat16


def build_nc(dt_mode: str = "bf16"):
    """Build the per-core Bass program. dt_mode in {"f32", "f32r", "bf16"}."""
    din = {"f32": f32, "f32r": f32r, "bf16": bf16}[dt_mode]
    ddram = din

    nc = bacc.Bacc("TRN2", target_bir_lowering=False, debug=False, num_devices=8)

    xT_d = nc.declare_dram_parameter("xT", [F, T], ddram, isOutput=False)
    wqT_d = nc.declare_dram_parameter("wqT", [F, CH], ddram, isOutput=False)
    wkT_d = nc.declare_dram_parameter("wkT", [F, CH], ddram, isOutput=False)
    wvT_d = nc.declare_dram_parameter("wvT", [F, CH], ddram, isOutput=False)
    wpT_d = nc.declare_dram_parameter("wpT", [CH, OUTC], ddram, isOutput=False)
    bq_d = nc.declare_dram_parameter("bq", [CH], f32, isOutput=False)
    bk_d = nc.declare_dram_parameter("bk", [CH], f32, isOutput=False)
    bv_d = nc.declare_dram_parameter("bv", [CH], f32, isOutput=False)
    bp_d = nc.declare_dram_parameter("bp", [OUTC], f32, isOutput=False)
    tri_d = nc.declare_dram_parameter("tri", [P, P], ddram, isOutput=False)
    out_d = nc.declare_dram_parameter("out", [T, OUTC], f32, isOutput=True)

    # softmax denominators: 32 chains of 512, bounced through DRAM so a
    # batched reciprocal can run across partitions
    l_d = nc.dram_tensor("l_i", [NCHAIN, GSZ], f32)
    r_d = nc.dram_tensor("r_i", [NCHAIN, GSZ], f32)

    add = mybir.AluOpType.add
    mult = mybir.AluOpType.mult

    with tile.TileContext(nc) as tc:
        from contextlib import ExitStack
        with ExitStack() as ctx:
            persist = ctx.enter_context(tc.tile_pool(name="persist", bufs=1))
            qkin = ctx.enter_context(tc.tile_pool(name="qkin", bufs=2))
            ptile = ctx.enter_context(tc.tile_pool(name="ptile", bufs=4))
            rtile = ctx.enter_context(tc.tile_pool(name="rtile", bufs=4))
            outs = ctx.enter_context(tc.tile_pool(name="outs", bufs=3))
            pps = ctx.enter_context(tc.tile_pool(name="pps", bufs=2, space="PSUM"))
            sps = ctx.enter_context(tc.tile_pool(name="sps", bufs=2, space="PSUM"))
            yps = ctx.enter_context(tc.tile_pool(name="yps", bufs=1, space="PSUM"))

            # persistent SBUF tensors
            xt_sb = persist.tile([P, NKO, T], din)
            v_sb = persist.tile([P, NTT, NH, D + 1], din)    # V + ones col
            yT_sb = persist.tile([P, NM, T], din)            # attn out, [ch, t]
            wq_sb = persist.tile([P, NKO, CH], din)
            wk_sb = persist.tile([P, NKO, CH], din)
            wv_sb = persist.tile([P, NKO, CH], din)
            wpT_sb = persist.tile([P, NM, OUTC], din)
            tri_sb = persist.tile([P, P], ddram)
            bq_sb = persist.tile([P, NM], f32)
            bk_sb = persist.tile([P, NM], f32)
            bv_bc = persist.tile([P, CH], f32)
            bp_bc = persist.tile([P, OUTC], f32)

            nc.sync.dma_start(out=wv_sb[:], in_=wvT_d[:].rearrange("(ko p) c -> p ko c", p=P))
            xT_r = xT_d[:].rearrange("(ko p) t -> p ko t", p=P)
            for ck in range(4):
                nc.sync.dma_start(out=xt_sb[:, :, ck * GSZ:(ck + 1) * GSZ],
                                  in_=xT_r[:, :, ck * GSZ:(ck + 1) * GSZ])
            nc.sync.dma_start(out=wq_sb[:], in_=wqT_d[:].rearrange("(ko p) c -> p ko c", p=P))
            nc.sync.dma_start(out=wk_sb[:], in_=wkT_d[:].rearrange("(ko p) c -> p ko c", p=P))
            nc.sync.dma_start(out=wpT_sb[:], in_=wpT_d[:].rearrange("(m p) o -> p m o", p=P))
            nc.sync.dma_start(out=tri_sb[:], in_=tri_d[:])
            nc.sync.dma_start(out=bq_sb[:], in_=bq_d[:].rearrange("(m p) -> p m", p=P))
            nc.sync.dma_start(out=bk_sb[:], in_=bk_d[:].rearrange("(m p) -> p m", p=P))
            nc.gpsimd.dma_start(out=bv_bc[:], in_=bv_d[None, :].to_broadcast((P, CH)))
            nc.gpsimd.dma_start(out=bp_bc[:], in_=bp_d[None, :].to_broadcast((P, OUTC)))
            ones_sb = persist.tile([P, NTT * NH], f32)
            nc.vector.memset(ones_sb[:], 1.0)
            nc.vector.tensor_copy(                           # ones columns in V
                out=v_sb[:, :, :, D],
                in_=ones_sb[:].rearrange("p (a b) -> p a b", b=NH))

            mask_eng = nc.gpsimd if hasattr(nc.gpsimd, "tensor_tensor") else nc.vector

            # ---------------- V for all channels (dense PE warm-up) ----------
            for tb in range(NTT):
                ps = pps.tile([P, CH], f32, tag="acc")
                for ko in range(NKO):
                    nc.tensor.matmul(
                        ps[:],
                        xt_sb[:, ko, tb * P:(tb + 1) * P],
                        wv_sb[:, ko, :],
                        start=(ko == 0), stop=(ko == NKO - 1),
                    )
                nc.vector.tensor_tensor(
                    out=v_sb[:, tb, :, 0:D],
                    in0=ps[:].rearrange("p (h d) -> p h d", d=D),
                    in1=bv_bc[:].rearrange("p (h d) -> p h d", d=D),
                    op=add,
                )

            # ------------- q/k projection units (emitted lazily) -------------
            def make_proj(gn):
                """Allocate qg/kg tiles for group gn; return (tiles, unit fns)."""
                qg_n = qkin.tile([P, T], din, tag="qg", name=f"qg{gn}")
                kg_n = qkin.tile([P, T], din, tag="kg", name=f"kg{gn}")
                units = []
                for (w_sb, b_sb, dst) in ((wq_sb, bq_sb, qg_n), (wk_sb, bk_sb, kg_n)):
                    for ck in range(4):
                        def unit(w_sb=w_sb, b_sb=b_sb, dst=dst, ck=ck):
                            ps = pps.tile([P, GSZ], f32, tag="acc")
                            for ko in range(NKO):
                                nc.tensor.matmul(
                                    ps[:],
                                    w_sb[:, ko, gn * P:(gn + 1) * P],
                                    xt_sb[:, ko, ck * GSZ:(ck + 1) * GSZ],
                                    start=(ko == 0), stop=(ko == NKO - 1),
                                )
                            nc.vector.tensor_scalar_add(
                                dst[:, ck * GSZ:(ck + 1) * GSZ], ps[:],
                                b_sb[:, gn:gn + 1])
                        units.append(unit)
            # interleaved: one unit every `step` attention j-iterations
                return (qg_n, kg_n), units

            (qg, kg), units0 = make_proj(0)
            for u in units0:
                u()

            # ---------------- per-group attention pipeline ----------------
            for g in range(NM):
                if g + 1 < NM:
                    (qg_next, kg_next), pend = make_proj(g + 1)
                else:
                    qg_next = kg_next = None
                    pend = []
                niter = 2 * (8 + 16)   # j-iterations in this group
                step = max(1, niter // (len(pend) + 1)) if pend else niter + 1
                it = 0

                for half in range(2):
                    gi0, gi1 = 2 * half, 2 * half + 1
                    g0, g1 = gi0 * GSZ, gi1 * GSZ
                    nblk0, nblk1 = (g0 + GSZ) // P, (g1 + GSZ) // P
                    for hh in range(2):
                        lo = hh * D
                        h = 2 * g + hh
                        y0 = yps.tile([D + 1, GSZ], f32, tag="y0",
                                      name=f"y0_{g}_{half}_{hh}")
                        y1 = yps.tile([D + 1, GSZ], f32, tag="y1",
                                      name=f"y1_{g}_{half}_{hh}")
                        for j in range(nblk1):
                            dlt0 = max(0, j * P - g0)
                            dlt1 = max(0, j * P - g1)
                            in0 = j < nblk0
                            off = dlt0 if in0 else GSZ + dlt1
                            s2 = sps.tile([P, 2 * GSZ], f32, tag="s")
                            if in0:
                                nc.tensor.matmul(
                                    s2[:, dlt0:GSZ],
                                    kg[lo:lo + D, j * P:(j + 1) * P],
                                    qg[lo:lo + D, g0 + dlt0:g0 + GSZ],
                                    start=True, stop=True, tile_position=(lo, 0),
                                )
                            nc.tensor.matmul(
                                s2[:, GSZ + dlt1:],
                                kg[lo:lo + D, j * P:(j + 1) * P],
                                qg[lo:lo + D, g1 + dlt1:g1 + GSZ],
                                start=True, stop=True, tile_position=(lo, 0),
                            )
                            p2 = ptile.tile([P, 2 * GSZ], din, tag="p")
                            nc.scalar.activation(
                                out=p2[:, off:], in_=s2[:, off:],
                                func=mybir.ActivationFunctionType.Exp)
                            # diagonal block: causal 0/1 mask
                            if in0 and j * P >= g0:
                                msk = slice(dlt0, dlt0 + P)
                            elif not in0:
                                msk = slice(GSZ + dlt1, GSZ + dlt1 + P)
                            else:
                                msk = None
                            if msk is not None:
                                mask_eng.tensor_tensor(
                                    out=p2[:, msk], in0=p2[:, msk],
                                    in1=tri_sb[:], op=mult)
                            if in0:
                                nc.tensor.matmul(
                                    y0[:, dlt0:],
                                    v_sb[:, j, h, :],
                                    p2[:, dlt0:GSZ],
                                    start=(j == 0), stop=(j == nblk0 - 1),
                                )
                            nc.tensor.matmul(
                                y1[:, dlt1:],
                                v_sb[:, j, h, :],
                                p2[:, GSZ + dlt1:],
                                start=(j == 0), stop=(j == nblk1 - 1),
                            )
                            it += 1
                            if pend and it % step == 0:
                                pend.pop(0)()
                        # stash unnormalized y and its denominator row
                        for gi, yt in ((gi0, y0), (gi1, y1)):
                            c = (g * NG + gi) * 2 + hh
                            nc.vector.tensor_copy(
                                out=yT_sb[lo:lo + D, g,
                                          gi * GSZ:(gi + 1) * GSZ],
                                in_=yt[0:D, :])
                            ls = rtile.tile([1, GSZ], f32, tag="ls")
                            nc.vector.tensor_copy(out=ls[:], in_=yt[D:D + 1, :])
                            nc.sync.dma_start(out=l_d[c:c + 1, :], in_=ls[:])
                while pend:
                    pend.pop(0)()

                # per-group batched reciprocal + normalize (overlaps g+1)
                lp = rtile.tile([NCHAIN, P], f32, tag="lp")
                rp = rtile.tile([NCHAIN, P], f32, tag="rp")
                cs = g * NG * 2
                nc.sync.dma_start(
                    out=lp[:], in_=l_d[cs:cs + 8, :].rearrange(
                        "c (a b) -> (c a) b", b=P))
                nc.vector.reciprocal(rp[:], lp[:])
                nc.sync.dma_start(
                    out=r_d[cs:cs + 8, :].rearrange("c (a b) -> (c a) b", b=P),
                    in_=rp[:])
                for gi in range(NG):
                    for hh in range(2):
                        c = (g * NG + gi) * 2 + hh
                        base = hh * D
                        rb = rtile.tile([P, GSZ], f32, tag="rb")
                        nc.sync.dma_start(
                            out=rb[base:base + D, :],
                            in_=r_d[c:c + 1, :].to_broadcast((D, GSZ)))
                        ysl = yT_sb[base:base + D, g, gi * GSZ:(gi + 1) * GSZ]
                        nc.vector.tensor_tensor(
                            out=ysl, in0=ysl, in1=rb[base:base + D, :], op=mult)

                qg, kg = qg_next, kg_next

            # ---------------- output projection ----------------
            for ts in range(NTT):
                for ih in range(OUTC // 512):
                    ps = pps.tile([P, 512], f32, tag="acc")
                    for co in range(NM):
                        nc.tensor.matmul(
                            ps[:],
                            yT_sb[:, co, ts * P:(ts + 1) * P],
                            wpT_sb[:, co, ih * 512:(ih + 1) * 512],
                            start=(co == 0), stop=(co == NM - 1),
                        )
                    ob = outs.tile([P, 512], f32, tag="ob")
                    nc.vector.tensor_tensor(
                        out=ob[:], in0=ps[:],
                        in1=bp_bc[:, ih * 512:(ih + 1) * 512], op=add)
                    nc.sync.dma_start(
                        out=out_d[ts * P:(ts + 1) * P, ih * 512:(ih + 1) * 512],
                        in_=ob[:])

    nc.compile()
    return nc


def make_in_maps(x, Wq, bq, Wk, bk, Wv, bv, Wp, bp, dt_mode="bf16"):
    """Shard full inputs into 8 per-core input maps."""
    import ml_dtypes
    npdt = ml_dtypes.bfloat16 if dt_mode == "bf16" else np.float32
    x = np.asarray(x, np.float32)
    scale = 1.0 / np.sqrt(D)
    tri = np.where(np.arange(P)[:, None] > np.arange(P)[None, :], 0.0, 1.0).astype(npdt)
    zeros_bp = np.zeros(OUTC, np.float32)
    in_maps = []
    for c in range(8):
        b, half = divmod(c, 2)
        sl = slice(half * CH, (half + 1) * CH)
        in_maps.append({
            "xT": np.ascontiguousarray(x[b].T).astype(npdt),
            "wqT": np.ascontiguousarray((np.asarray(Wq, np.float32)[sl] * scale).T).astype(npdt),
            "wkT": np.ascontiguousarray(np.asarray(Wk, np.float32)[sl].T).astype(npdt),
            "wvT": np.ascontiguousarray(np.asarray(Wv, np.float32)[sl].T).astype(npdt),
            "wpT": np.ascontiguousarray(np.asarray(Wp, np.float32)[:, sl].T).astype(npdt),
            "bq": (np.asarray(bq, np.float32)[sl] * scale).copy(),
            "bk": np.asarray(bk, np.float32)[sl].copy(),
            "bv": np.asarray(bv, np.float32)[sl].copy(),
            "bp": np.asarray(bp, np.float32).copy() if half == 0 else zeros_bp,
            "tri": tri,
        })
    return in_maps


def combine(results):
    """results: list of 8 dicts with 'out' [T, OUTC] partials -> [4, T, OUTC]."""
    return np.stack([results[2 * b]["out"] + results[2 * b + 1]["out"]
                     for b in range(4)]).astype(np.float32)


# ----------------------------------------------------------------------------
# Harness entry point: full inputs in, full output out.
# ----------------------------------------------------------------------------
_NC_CACHE = {}


def _get_nc(dt_mode):
    if dt_mode not in _NC_CACHE:
        _NC_CACHE[dt_mode] = build_nc(dt_mode)
    return _NC_CACHE[dt_mode]


def kernel(x, Wq, bq, Wk, bk, Wv, bv, Wp, bp):
    from concourse.bass_utils import run_bass_kernel_spmd
    dt_mode = "bf16"
    nc = _get_nc(dt_mode)
    in_maps = make_in_maps(x, Wq, bq, Wk, bk, Wv, bv, Wp, bp, dt_mode)
    res = run_bass_kernel_spmd(nc, in_maps, list(range(8)))
    return combine(res.results)


# revision 7
# speedup vs baseline: 1.0793x; 1.0793x over previous
import sys
for _p in ("/opt/trn_rl_repo",):
    if _p not in sys.path:
        sys.path.insert(0, _p)
"""Causal self-attention TRN2 kernel, 8-core SPMD — fused pipeline v2.

Sharding: core c handles batch b = c//2 and head-half h = c%2 (8 of 16 heads,
i.e. 512 of 1024 q/k/v channels). Host sums the two output-projection partials
per batch and concatenates batches.

v2 layout (vs v1 phase-separated):
  - x resident in SBUF whole kernel; V computed first (dense PE warm-up for
    the HAM clock gate), then per channel-group g (= head pair): q/k projected
    straight into SBUF (no DRAM bounce), attention for g immediately after.
  - Projection matmuls for group g+1 are interleaved into group g's attention
    j-loop so the PE has dense work while ACT (exp) paces the softmax.
  - hh processed serially (not tile-position-packed) to fit PSUM in 8 banks:
    S double-buffer 2x[128,1024] (4) + y accum 2x[65,512] (2) + shared
    accumulation ring 2x[128,512] (2).
  - Softmax denominators via ones-column in V (y row 64); per-g batched
    reciprocal bounced through DRAM (cross-partition gather), normalize
    overlapped under the next group's attention.
"""

import numpy as np
import concourse.bass as bass
import concourse.mybir as mybir
import concourse.tile as tile
from concourse import bacc

P = 128
T = 2048          # tokens per batch
F = 1024          # model dim (projection contraction)
CH = 512          # per-core q/k/v channels (8 heads x 64)
NH = 8            # heads per core
D = 64
NKO = F // P      # 8
NM = CH // P      # 4 channel tiles (= head pairs)
GSZ = 512         # tq group size
NG = T // GSZ     # 4
NTT = T // P      # 16
OUTC = 1024
NCHAIN = NM * NG * 2   # 32 softmax-denominator chains

f32 = mybir.dt.float32
f32r = mybir.dt.float32r
bf16 = mybir.dt.bfloat16


def build_nc(dt_mode: str = "bf16"):
    """Build the per-core Bass program. dt_mode in {"f32", "f32r", "bf16"}."""
    din = {"f32": f32, "f32r": f32r, "bf16": bf16}[dt_mode]
    ddram = din

    nc = bacc.Bacc("TRN2", target_bir_lowering=False, debug=False, num_devices=8)

    xT_d = nc.declare_dram_parameter("xT", [F, T], ddram, isOutput=False)
    wqT_d = nc.declare_dram_parameter("wqT", [F, CH], ddram, isOutput=False)
    wkT_d = nc.declare_dram_parameter("wkT", [F, CH], ddram, isOutput=False)
    wvT_d = nc.declare_dram_parameter("wvT", [F, CH], ddram, isOutput=False)
    wpT_d = nc.declare_dram_parameter("wpT", [CH, OUTC], ddram, isOutput=False)
    bq_d = nc.declare_dram_parameter("bq", [CH], f32, isOutput=False)
    bk_d = nc.declare_dram_parameter("bk", [CH], f32, isOutput=False)
    bv_d = nc.declare_dram_parameter("bv", [CH], f32, isOutput=False)
    bp_d = nc.declare_dram_parameter("bp", [OUTC], f32, isOutput=False)
    tri_d = nc.declare_dram_parameter("tri", [P, P], ddram, isOutput=False)
    out_d = nc.declare_dram_parameter("out", [T, OUTC], f32, isOutput=True)

    # softmax denominators: 32 chains of 512, bounced through DRAM so a
    # batched reciprocal can run across partitions
    l_d = nc.dram_tensor("l_i", [NCHAIN, GSZ], f32)
    r_d = nc.dram_tensor("r_i", [NCHAIN, GSZ], f32)

    add = mybir.AluOpType.add
    mult = mybir.AluOpType.mult

    with tile.TileContext(nc) as tc:
        from contextlib import ExitStack
        with ExitStack() as ctx:
            persist = ctx.enter_context(tc.tile_pool(name="persist", bufs=1))
            qkin = ctx.enter_context(tc.tile_pool(name="qkin", bufs=2))
            ptile = ctx.enter_context(tc.tile_pool(name="ptile", bufs=4))
            rtile = ctx.enter_context(tc.tile_pool(name="rtile", bufs=4))
            outs = ctx.enter_context(tc.tile_pool(name="outs", bufs=3))
            pps = ctx.enter_context(tc.tile_pool(name="pps", bufs=2, space="PSUM"))
            sps = ctx.enter_context(tc.tile_pool(name="sps", bufs=2, space="PSUM"))
            yps = ctx.enter_context(tc.tile_pool(name="yps", bufs=1, space="PSUM"))

            # persistent SBUF tensors
            xt_sb = persist.tile([P, NKO, T], din)
            v_sb = persist.tile([P, NTT, NH, D + 1], din)    # V + ones col
            yT_sb = persist.tile([P, NM, T], din)            # attn out, [ch, t]
            wq_sb = persist.tile([P, NKO, CH], din)
            wk_sb = persist.tile([P, NKO, CH], din)
            wv_sb = persist.tile([P, NKO, CH], din)
            wpT_sb = persist.tile([P, NM, OUTC], din)
            tri_sb = persist.tile([P, P], ddram)
            bq_sb = persist.tile([P, NM], f32)
            bk_sb = persist.tile([P, NM], f32)
            bv_bc = persist.tile([P, CH], f32)
            bp_bc = persist.tile([P, OUTC], f32)

            nc.sync.dma_start(out=wv_sb[:], in_=wvT_d[:].rearrange("(ko p) c -> p ko c", p=P))
            xT_r = xT_d[:].rearrange("(ko p) t -> p ko t", p=P)
            for ck, eng in enumerate((nc.sync, nc.scalar, nc.gpsimd, nc.scalar)):
                eng.dma_start(out=xt_sb[:, :, ck * GSZ:(ck + 1) * GSZ],
                              in_=xT_r[:, :, ck * GSZ:(ck + 1) * GSZ])
            nc.sync.dma_start(out=wq_sb[:], in_=wqT_d[:].rearrange("(ko p) c -> p ko c", p=P))
            nc.sync.dma_start(out=wk_sb[:], in_=wkT_d[:].rearrange("(ko p) c -> p ko c", p=P))
            nc.sync.dma_start(out=wpT_sb[:], in_=wpT_d[:].rearrange("(m p) o -> p m o", p=P))
            nc.sync.dma_start(out=tri_sb[:], in_=tri_d[:])
            nc.sync.dma_start(out=bq_sb[:], in_=bq_d[:].rearrange("(m p) -> p m", p=P))
            nc.sync.dma_start(out=bk_sb[:], in_=bk_d[:].rearrange("(m p) -> p m", p=P))
            nc.gpsimd.dma_start(out=bv_bc[:], in_=bv_d[None, :].to_broadcast((P, CH)))
            nc.gpsimd.dma_start(out=bp_bc[:], in_=bp_d[None, :].to_broadcast((P, OUTC)))
            ones_sb = persist.tile([P, NTT * NH], f32)
            nc.vector.memset(ones_sb[:], 1.0)
            nc.vector.tensor_copy(                           # ones columns in V
                out=v_sb[:, :, :, D],
                in_=ones_sb[:].rearrange("p (a b) -> p a b", b=NH))

            mask_eng = nc.vector

            # ---------------- V for all channels (dense PE warm-up) ----------
            for tb in range(NTT):
                ps = pps.tile([P, CH], f32, tag="acc")
                for ko in range(NKO):
                    nc.tensor.matmul(
                        ps[:],
                        xt_sb[:, ko, tb * P:(tb + 1) * P],
                        wv_sb[:, ko, :],
                        start=(ko == 0), stop=(ko == NKO - 1),
                    )
                nc.vector.tensor_tensor(
                    out=v_sb[:, tb, :, 0:D],
                    in0=ps[:].rearrange("p (h d) -> p h d", d=D),
                    in1=bv_bc[:].rearrange("p (h d) -> p h d", d=D),
                    op=add,
                )

            # ------------- q/k projection units (emitted lazily) -------------
            def make_proj(gn):
                """Allocate qg/kg tiles for group gn; return (tiles, unit fns)."""
                qg_n = qkin.tile([P, T], din, tag="qg", name=f"qg{gn}")
                kg_n = qkin.tile([P, T], din, tag="kg", name=f"kg{gn}")
                units = []
                for (w_sb, b_sb, dst) in ((wq_sb, bq_sb, qg_n), (wk_sb, bk_sb, kg_n)):
                    for ck in range(4):
                        def unit(w_sb=w_sb, b_sb=b_sb, dst=dst, ck=ck):
                            ps = pps.tile([P, GSZ], f32, tag="acc")
                            for ko in range(NKO):
                                nc.tensor.matmul(
                                    ps[:],
                                    w_sb[:, ko, gn * P:(gn + 1) * P],
                                    xt_sb[:, ko, ck * GSZ:(ck + 1) * GSZ],
                                    start=(ko == 0), stop=(ko == NKO - 1),
                                )
                            nc.vector.tensor_scalar_add(
                                dst[:, ck * GSZ:(ck + 1) * GSZ], ps[:],
                                b_sb[:, gn:gn + 1])
                        units.append(unit)
            # interleaved: one unit every `step` attention j-iterations
                return (qg_n, kg_n), units

            (qg, kg), units0 = make_proj(0)
            for u in units0:
                u()

            # ------------- output-projection units (emitted lazily) -----------
            def ph3_unit(ts, ih):
                def unit():
                    ps = pps.tile([P, 512], f32, tag="acc")
                    for co in range(NM):
                        nc.tensor.matmul(
                            ps[:],
                            yT_sb[:, co, ts * P:(ts + 1) * P],
                            wpT_sb[:, co, ih * 512:(ih + 1) * 512],
                            start=(co == 0), stop=(co == NM - 1),
                        )
                    ob = outs.tile([P, 512], f32, tag="ob")
                    nc.vector.tensor_tensor(
                        out=ob[:], in0=ps[:],
                        in1=bp_bc[:, ih * 512:(ih + 1) * 512], op=add)
                    nc.sync.dma_start(
                        out=out_d[ts * P:(ts + 1) * P, ih * 512:(ih + 1) * 512],
                        in_=ob[:])
                return unit

            ph3 = [ph3_unit(ts, ih) for ts in range(NTT) for ih in range(2)]

            # per-(group, half) denominator reciprocal + normalize
            def epilogue(g, half):
                cs = g * 8 + half * 4
                lp = rtile.tile([16, P], f32, tag="lp")
                rp = rtile.tile([16, P], f32, tag="rp")
                nc.sync.dma_start(
                    out=lp[:], in_=l_d[cs:cs + 4, :].rearrange(
                        "c (a b) -> (c a) b", b=P))
                nc.vector.reciprocal(rp[:], lp[:])
                nc.sync.dma_start(
                    out=r_d[cs:cs + 4, :].rearrange("c (a b) -> (c a) b", b=P),
                    in_=rp[:])
                for gi in (2 * half, 2 * half + 1):
                    for hh in range(2):
                        c = (g * NG + gi) * 2 + hh
                        base = hh * D
                        rb = rtile.tile([P, GSZ], f32, tag="rb")
                        nc.sync.dma_start(
                            out=rb[base:base + D, :],
                            in_=r_d[c:c + 1, :].to_broadcast((D, GSZ)))
                        ysl = yT_sb[base:base + D, g, gi * GSZ:(gi + 1) * GSZ]
                        nc.vector.tensor_tensor(
                            out=ysl, in0=ysl, in1=rb[base:base + D, :], op=mult)

            # ---------------- per-group attention pipeline ----------------
            for g in range(NM):
                if g + 1 < NM:
                    (qg_next, kg_next), pend = make_proj(g + 1)
                else:
                    qg_next = kg_next = None
                    pend = []
                niter = 2 * (8 + 16)   # j-iterations in this group
                step = max(1, niter // (len(pend) + 1)) if pend else niter + 1
                it = 0

                for half in range(2):
                    if g == NM - 1 and half == 1:
                        # fill the PE during the last group's second half with
                        # output-projection work for already-normalized tokens
                        pend = ph3[:16]
                        step, it = 2, 0
                    gi0, gi1 = 2 * half, 2 * half + 1
                    g0, g1 = gi0 * GSZ, gi1 * GSZ
                    nblk0, nblk1 = (g0 + GSZ) // P, (g1 + GSZ) // P
                    for hh in range(2):
                        lo = hh * D
                        h = 2 * g + hh
                        y0 = yps.tile([D + 1, GSZ], f32, tag="y0",
                                      name=f"y0_{g}_{half}_{hh}")
                        y1 = yps.tile([D + 1, GSZ], f32, tag="y1",
                                      name=f"y1_{g}_{half}_{hh}")
                        for j in range(nblk1):
                            dlt0 = max(0, j * P - g0)
                            dlt1 = max(0, j * P - g1)
                            in0 = j < nblk0
                            off = dlt0 if in0 else GSZ + dlt1
                            s2 = sps.tile([P, 2 * GSZ], f32, tag="s")
                            if in0:
                                nc.tensor.matmul(
                                    s2[:, dlt0:GSZ],
                                    kg[lo:lo + D, j * P:(j + 1) * P],
                                    qg[lo:lo + D, g0 + dlt0:g0 + GSZ],
                                    start=True, stop=True, tile_position=(lo, 0),
                                )
                            nc.tensor.matmul(
                                s2[:, GSZ + dlt1:],
                                kg[lo:lo + D, j * P:(j + 1) * P],
                                qg[lo:lo + D, g1 + dlt1:g1 + GSZ],
                                start=True, stop=True, tile_position=(lo, 0),
                            )
                            p2 = ptile.tile([P, 2 * GSZ], din, tag="p")
                            nc.scalar.activation(
                                out=p2[:, off:], in_=s2[:, off:],
                                func=mybir.ActivationFunctionType.Exp)
                            # diagonal block: causal 0/1 mask
                            if in0 and j * P >= g0:
                                msk = slice(dlt0, dlt0 + P)
                            elif not in0:
                                msk = slice(GSZ + dlt1, GSZ + dlt1 + P)
                            else:
                                msk = None
                            if msk is not None:
                                mask_eng.tensor_tensor(
                                    out=p2[:, msk], in0=p2[:, msk],
                                    in1=tri_sb[:], op=mult)
                            if in0:
                                nc.tensor.matmul(
                                    y0[:, dlt0:],
                                    v_sb[:, j, h, :],
                                    p2[:, dlt0:GSZ],
                                    start=(j == 0), stop=(j == nblk0 - 1),
                                )
                            nc.tensor.matmul(
                                y1[:, dlt1:],
                                v_sb[:, j, h, :],
                                p2[:, GSZ + dlt1:],
                                start=(j == 0), stop=(j == nblk1 - 1),
                            )
                            it += 1
                            if pend and it % step == 0:
                                pend.pop(0)()
                        # stash unnormalized y and its denominator row
                        for gi, yt in ((gi0, y0), (gi1, y1)):
                            c = (g * NG + gi) * 2 + hh
                            nc.vector.tensor_copy(
                                out=yT_sb[lo:lo + D, g,
                                          gi * GSZ:(gi + 1) * GSZ],
                                in_=yt[0:D, :])
                            ls = rtile.tile([1, GSZ], f32, tag="ls")
                            nc.vector.tensor_copy(out=ls[:], in_=yt[D:D + 1, :])
                            nc.sync.dma_start(out=l_d[c:c + 1, :], in_=ls[:])
                    epilogue(g, half)
                while pend:
                    pend.pop(0)()
                if g == NM - 1:
                    for u in ph3[16:]:
                        u()

                qg, kg = qg_next, kg_next

    nc.compile()
    return nc


def make_in_maps(x, Wq, bq, Wk, bk, Wv, bv, Wp, bp, dt_mode="bf16"):
    """Shard full inputs into 8 per-core input maps."""
    import ml_dtypes
    npdt = ml_dtypes.bfloat16 if dt_mode == "bf16" else np.float32
    x = np.asarray(x, np.float32)
    scale = 1.0 / np.sqrt(D)
    tri = np.where(np.arange(P)[:, None] > np.arange(P)[None, :], 0.0, 1.0).astype(npdt)
    zeros_bp = np.zeros(OUTC, np.float32)
    in_maps = []
    for c in range(8):
        b, half = divmod(c, 2)
        sl = slice(half * CH, (half + 1) * CH)
        in_maps.append({
            "xT": np.ascontiguousarray(x[b].T).astype(npdt),
            "wqT": np.ascontiguousarray((np.asarray(Wq, np.float32)[sl] * scale).T).astype(npdt),
            "wkT": np.ascontiguousarray(np.asarray(Wk, np.float32)[sl].T).astype(npdt),
            "wvT": np.ascontiguousarray(np.asarray(Wv, np.float32)[sl].T).astype(npdt),
            "wpT": np.ascontiguousarray(np.asarray(Wp, np.float32)[:, sl].T).astype(npdt),
            "bq": (np.asarray(bq, np.float32)[sl] * scale).copy(),
            "bk": np.asarray(bk, np.float32)[sl].copy(),
            "bv": np.asarray(bv, np.float32)[sl].copy(),
            "bp": np.asarray(bp, np.float32).copy() if half == 0 else zeros_bp,
            "tri": tri,
        })
    return in_maps


def combine(results):
    """results: list of 8 dicts with 'out' [T, OUTC] partials -> [4, T, OUTC]."""
    return np.stack([results[2 * b]["out"] + results[2 * b + 1]["out"]
                     for b in range(4)]).astype(np.float32)


# ----------------------------------------------------------------------------
# Harness entry point: full inputs in, full output out.
# ----------------------------------------------------------------------------
_NC_CACHE = {}


def _get_nc(dt_mode):
    if dt_mode not in _NC_CACHE:
        _NC_CACHE[dt_mode] = build_nc(dt_mode)
    return _NC_CACHE[dt_mode]


def kernel(x, Wq, bq, Wk, bk, Wv, bv, Wp, bp):
    from concourse.bass_utils import run_bass_kernel_spmd
    dt_mode = "bf16"
    nc = _get_nc(dt_mode)
    in_maps = make_in_maps(x, Wq, bq, Wk, bk, Wv, bv, Wp, bp, dt_mode)
    res = run_bass_kernel_spmd(nc, in_maps, list(range(8)))
    return combine(res.results)


# revision 10
# speedup vs baseline: 1.1014x; 1.0205x over previous
import sys
for _p in ("/opt/trn_rl_repo",):
    if _p not in sys.path:
        sys.path.insert(0, _p)
"""Causal self-attention TRN2 kernel, 8-core SPMD — fused pipeline v2.

Sharding: core c handles batch b = c//2 and head-half h = c%2 (8 of 16 heads,
i.e. 512 of 1024 q/k/v channels). Host sums the two output-projection partials
per batch and concatenates batches.

v2 layout (vs v1 phase-separated):
  - x resident in SBUF whole kernel; V computed first (dense PE warm-up for
    the HAM clock gate), then per channel-group g (= head pair): q/k projected
    straight into SBUF (no DRAM bounce), attention for g immediately after.
  - Projection matmuls for group g+1 are interleaved into group g's attention
    j-loop so the PE has dense work while ACT (exp) paces the softmax.
  - hh processed serially (not tile-position-packed) to fit PSUM in 8 banks:
    S double-buffer 2x[128,1024] (4) + y accum 2x[65,512] (2) + shared
    accumulation ring 2x[128,512] (2).
  - Softmax denominators via ones-column in V (y row 64); per-g batched
    reciprocal bounced through DRAM (cross-partition gather), normalize
    overlapped under the next group's attention.
"""

import numpy as np
import concourse.bass as bass
import concourse.mybir as mybir
import concourse.tile as tile
from concourse import bacc

P = 128
T = 2048          # tokens per batch
F = 1024          # model dim (projection contraction)
CH = 512          # per-core q/k/v channels (8 heads x 64)
NH = 8            # heads per core
D = 64
NKO = F // P      # 8
NM = CH // P      # 4 channel tiles (= head pairs)
GSZ = 512         # tq group size
NG = T // GSZ     # 4
NTT = T // P      # 16
OUTC = 1024
NCHAIN = NM * NG * 2   # 32 softmax-denominator chains

f32 = mybir.dt.float32
f32r = mybir.dt.float32r
bf16 = mybir.dt.bfloat16


def build_nc(dt_mode: str = "bf16"):
    """Build the per-core Bass program. dt_mode in {"f32", "f32r", "bf16"}."""
    din = {"f32": f32, "f32r": f32r, "bf16": bf16}[dt_mode]
    ddram = din

    nc = bacc.Bacc("TRN2", target_bir_lowering=False, debug=False, num_devices=8)

    xT_d = nc.declare_dram_parameter("xT", [F, T], ddram, isOutput=False)
    wqT_d = nc.declare_dram_parameter("wqT", [F, CH], ddram, isOutput=False)
    wkT_d = nc.declare_dram_parameter("wkT", [F, CH], ddram, isOutput=False)
    wvT_d = nc.declare_dram_parameter("wvT", [F, CH], ddram, isOutput=False)
    wpT_d = nc.declare_dram_parameter("wpT", [CH, OUTC], ddram, isOutput=False)
    bq_d = nc.declare_dram_parameter("bq", [CH], f32, isOutput=False)
    bk_d = nc.declare_dram_parameter("bk", [CH], f32, isOutput=False)
    bv_d = nc.declare_dram_parameter("bv", [CH], f32, isOutput=False)
    bp_d = nc.declare_dram_parameter("bp", [OUTC], f32, isOutput=False)
    tri_d = nc.declare_dram_parameter("tri", [P, P], ddram, isOutput=False)
    out_d = nc.declare_dram_parameter("out", [T, OUTC], f32, isOutput=True)

    # softmax denominators: 32 chains of 512, bounced through DRAM so a
    # batched reciprocal can run across partitions
    l_d = nc.dram_tensor("l_i", [NCHAIN, GSZ], f32)
    r_d = nc.dram_tensor("r_i", [NCHAIN, GSZ], f32)

    add = mybir.AluOpType.add
    mult = mybir.AluOpType.mult

    with tile.TileContext(nc) as tc:
        from contextlib import ExitStack
        with ExitStack() as ctx:
            persist = ctx.enter_context(tc.tile_pool(name="persist", bufs=1))
            qkin = ctx.enter_context(tc.tile_pool(name="qkin", bufs=2))
            ptile = ctx.enter_context(tc.tile_pool(name="ptile", bufs=4))
            rtile = ctx.enter_context(tc.tile_pool(name="rtile", bufs=4))
            outs = ctx.enter_context(tc.tile_pool(name="outs", bufs=3))
            pps = ctx.enter_context(tc.tile_pool(name="pps", bufs=2, space="PSUM"))
            sps = ctx.enter_context(tc.tile_pool(name="sps", bufs=2, space="PSUM"))
            yps = ctx.enter_context(tc.tile_pool(name="yps", bufs=1, space="PSUM"))

            # persistent SBUF tensors
            xt_sb = persist.tile([P, NKO, T], din)
            v_sb = persist.tile([P, NTT, NH, D + 1], din)    # V + ones col
            yT_sb = persist.tile([P, NM, T], din)            # attn out, [ch, t]
            wq_sb = persist.tile([P, NKO, CH], din)
            wk_sb = persist.tile([P, NKO, CH], din)
            wv_sb = persist.tile([P, NKO, CH], din)
            wpT_sb = persist.tile([P, NM, OUTC], din)
            tri_sb = persist.tile([P, P], ddram)
            bq_sb = persist.tile([P, NM], f32)
            bk_sb = persist.tile([P, NM], f32)
            bv_bc = persist.tile([P, CH], f32)
            bp_bc = persist.tile([P, OUTC], f32)

            # critical-path DMAs first: V needs x chunk 0 + Wv immediately
            xT_r = xT_d[:].rearrange("(ko p) t -> p ko t", p=P)
            nc.sync.dma_start(out=xt_sb[:, :, 0:GSZ], in_=xT_r[:, :, 0:GSZ])
            nc.gpsimd.dma_start(out=wv_sb[:], in_=wvT_d[:].rearrange("(ko p) c -> p ko c", p=P))
            for ck in range(1, 4):
                nc.scalar.dma_start(out=xt_sb[:, :, ck * GSZ:(ck + 1) * GSZ],
                                    in_=xT_r[:, :, ck * GSZ:(ck + 1) * GSZ])
            nc.sync.dma_start(out=wq_sb[:], in_=wqT_d[:].rearrange("(ko p) c -> p ko c", p=P))
            nc.sync.dma_start(out=wk_sb[:], in_=wkT_d[:].rearrange("(ko p) c -> p ko c", p=P))
            nc.sync.dma_start(out=tri_sb[:], in_=tri_d[:])
            nc.sync.dma_start(out=bq_sb[:], in_=bq_d[:].rearrange("(m p) -> p m", p=P))
            nc.sync.dma_start(out=bk_sb[:], in_=bk_d[:].rearrange("(m p) -> p m", p=P))
            nc.gpsimd.dma_start(out=bv_bc[:], in_=bv_d[None, :].to_broadcast((P, CH)))
            nc.gpsimd.dma_start(out=bp_bc[:], in_=bp_d[None, :].to_broadcast((P, OUTC)))
            nc.gpsimd.dma_start(out=wpT_sb[:], in_=wpT_d[:].rearrange("(m p) o -> p m o", p=P))
            ones_sb = persist.tile([P, NTT * NH], f32)
            nc.vector.memset(ones_sb[:], 1.0)
            nc.vector.tensor_copy(                           # ones columns in V
                out=v_sb[:, :, :, D],
                in_=ones_sb[:].rearrange("p (a b) -> p a b", b=NH))

            mask_eng = nc.vector

            # ---------------- V for all channels (dense PE warm-up) ----------
            for tb in range(NTT):
                ps = pps.tile([P, CH], f32, tag="acc")
                for ko in range(NKO):
                    nc.tensor.matmul(
                        ps[:],
                        xt_sb[:, ko, tb * P:(tb + 1) * P],
                        wv_sb[:, ko, :],
                        start=(ko == 0), stop=(ko == NKO - 1),
                    )
                nc.vector.tensor_tensor(
                    out=v_sb[:, tb, :, 0:D],
                    in0=ps[:].rearrange("p (h d) -> p h d", d=D),
                    in1=bv_bc[:].rearrange("p (h d) -> p h d", d=D),
                    op=add,
                )

            # ------------- q/k projection units (emitted lazily) -------------
            def make_proj(gn):
                """Allocate qg/kg tiles for group gn; return (tiles, unit fns)."""
                qg_n = qkin.tile([P, T], din, tag="qg", name=f"qg{gn}")
                kg_n = qkin.tile([P, T], din, tag="kg", name=f"kg{gn}")
                units = []
                for (w_sb, b_sb, dst) in ((wq_sb, bq_sb, qg_n), (wk_sb, bk_sb, kg_n)):
                    for ck in range(4):
                        def unit(w_sb=w_sb, b_sb=b_sb, dst=dst, ck=ck):
                            ps = pps.tile([P, GSZ], f32, tag="acc")
                            for ko in range(NKO):
                                nc.tensor.matmul(
                                    ps[:],
                                    w_sb[:, ko, gn * P:(gn + 1) * P],
                                    xt_sb[:, ko, ck * GSZ:(ck + 1) * GSZ],
                                    start=(ko == 0), stop=(ko == NKO - 1),
                                )
                            nc.vector.tensor_scalar_add(
                                dst[:, ck * GSZ:(ck + 1) * GSZ], ps[:],
                                b_sb[:, gn:gn + 1])
                        units.append(unit)
            # interleaved: one unit every `step` attention j-iterations
                return (qg_n, kg_n), units

            (qg, kg), units0 = make_proj(0)
            for u in units0:
                u()

            # ------------- output-projection units (emitted lazily) -----------
            _oq = [nc.sync, nc.scalar, nc.gpsimd]

            def ph3_unit(ts, ih):
                def unit():
                    ps = pps.tile([P, 512], f32, tag="acc")
                    for co in range(NM):
                        nc.tensor.matmul(
                            ps[:],
                            yT_sb[:, co, ts * P:(ts + 1) * P],
                            wpT_sb[:, co, ih * 512:(ih + 1) * 512],
                            start=(co == 0), stop=(co == NM - 1),
                        )
                    ob = outs.tile([P, 512], f32, tag="ob")
                    nc.vector.tensor_tensor(
                        out=ob[:], in0=ps[:],
                        in1=bp_bc[:, ih * 512:(ih + 1) * 512], op=add)
                    _oq[(ts * 2 + ih) % 3].dma_start(
                        out=out_d[ts * P:(ts + 1) * P, ih * 512:(ih + 1) * 512],
                        in_=ob[:])
                return unit

            ph3 = [ph3_unit(ts, ih) for ts in range(NTT) for ih in range(2)]

            # per-(group, half) denominator reciprocal + normalize
            def epilogue(g, half):
                cs = g * 8 + half * 4
                lp = rtile.tile([16, P], f32, tag="lp")
                rp = rtile.tile([16, P], f32, tag="rp")
                nc.sync.dma_start(
                    out=lp[:], in_=l_d[cs:cs + 4, :].rearrange(
                        "c (a b) -> (c a) b", b=P))
                nc.vector.reciprocal(rp[:], lp[:])
                nc.sync.dma_start(
                    out=r_d[cs:cs + 4, :].rearrange("c (a b) -> (c a) b", b=P),
                    in_=rp[:])
                for gi in (2 * half, 2 * half + 1):
                    for hh in range(2):
                        c = (g * NG + gi) * 2 + hh
                        base = hh * D
                        rb = rtile.tile([P, GSZ], f32, tag="rb")
                        nc.sync.dma_start(
                            out=rb[base:base + D, :],
                            in_=r_d[c:c + 1, :].to_broadcast((D, GSZ)))
                        ysl = yT_sb[base:base + D, g, gi * GSZ:(gi + 1) * GSZ]
                        nc.vector.tensor_tensor(
                            out=ysl, in0=ysl, in1=rb[base:base + D, :], op=mult)

            # ---------------- per-group attention pipeline ----------------
            for g in range(NM):
                if g + 1 < NM:
                    (qg_next, kg_next), pend = make_proj(g + 1)
                else:
                    qg_next = kg_next = None
                    pend = []
                niter = 2 * (8 + 16)   # j-iterations in this group
                step = max(1, niter // (len(pend) + 1)) if pend else niter + 1
                it = 0

                for half in range(2):
                    if g == NM - 1 and half == 1:
                        # fill the PE during the last group's second half with
                        # output-projection work for already-normalized tokens
                        pend = ph3[:16]
                        step, it = 2, 0
                    gi0, gi1 = 2 * half, 2 * half + 1
                    g0, g1 = gi0 * GSZ, gi1 * GSZ
                    nblk0, nblk1 = (g0 + GSZ) // P, (g1 + GSZ) // P
                    for hh in range(2):
                        lo = hh * D
                        h = 2 * g + hh
                        y0 = yps.tile([D + 1, GSZ], f32, tag="y0",
                                      name=f"y0_{g}_{half}_{hh}")
                        y1 = yps.tile([D + 1, GSZ], f32, tag="y1",
                                      name=f"y1_{g}_{half}_{hh}")
                        # PV is software-pipelined one iteration behind S/exp
                        # so the PE never head-of-line-blocks waiting on exp.
                        pv_pending = None

                        def emit_pv(j, p2, dlt0, dlt1, in0):
                            if in0:
                                nc.tensor.matmul(
                                    y0[:, dlt0:],
                                    v_sb[:, j, h, :],
                                    p2[:, dlt0:GSZ],
                                    start=(j == 0), stop=(j == nblk0 - 1),
                                )
                            nc.tensor.matmul(
                                y1[:, dlt1:],
                                v_sb[:, j, h, :],
                                p2[:, GSZ + dlt1:],
                                start=(j == 0), stop=(j == nblk1 - 1),
                            )

                        for j in range(nblk1):
                            dlt0 = max(0, j * P - g0)
                            dlt1 = max(0, j * P - g1)
                            in0 = j < nblk0
                            off = dlt0 if in0 else GSZ + dlt1
                            s2 = sps.tile([P, 2 * GSZ], f32, tag="s")
                            if in0:
                                nc.tensor.matmul(
                                    s2[:, dlt0:GSZ],
                                    kg[lo:lo + D, j * P:(j + 1) * P],
                                    qg[lo:lo + D, g0 + dlt0:g0 + GSZ],
                                    start=True, stop=True, tile_position=(lo, 0),
                                )
                            nc.tensor.matmul(
                                s2[:, GSZ + dlt1:],
                                kg[lo:lo + D, j * P:(j + 1) * P],
                                qg[lo:lo + D, g1 + dlt1:g1 + GSZ],
                                start=True, stop=True, tile_position=(lo, 0),
                            )
                            p2 = ptile.tile([P, 2 * GSZ], din, tag="p")
                            nc.scalar.activation(
                                out=p2[:, off:], in_=s2[:, off:],
                                func=mybir.ActivationFunctionType.Exp)
                            # diagonal block: causal 0/1 mask
                            if in0 and j * P >= g0:
                                msk = slice(dlt0, dlt0 + P)
                            elif not in0:
                                msk = slice(GSZ + dlt1, GSZ + dlt1 + P)
                            else:
                                msk = None
                            if msk is not None:
                                mask_eng.tensor_tensor(
                                    out=p2[:, msk], in0=p2[:, msk],
                                    in1=tri_sb[:], op=mult)
                            if pv_pending is not None:
                                emit_pv(*pv_pending)
                            pv_pending = (j, p2, dlt0, dlt1, in0)
                            it += 1
                            if pend and it % step == 0:
                                pend.pop(0)()
                        emit_pv(*pv_pending)
                        # stash unnormalized y and its denominator row
                        for gi, yt in ((gi0, y0), (gi1, y1)):
                            c = (g * NG + gi) * 2 + hh
                            nc.vector.tensor_copy(
                                out=yT_sb[lo:lo + D, g,
                                          gi * GSZ:(gi + 1) * GSZ],
                                in_=yt[0:D, :])
                            ls = rtile.tile([1, GSZ], f32, tag="ls")
                            nc.vector.tensor_copy(out=ls[:], in_=yt[D:D + 1, :])
                            nc.sync.dma_start(out=l_d[c:c + 1, :], in_=ls[:])
                    epilogue(g, half)
                while pend:
                    pend.pop(0)()
                if g == NM - 1:
                    for u in ph3[16:]:
                        u()

                qg, kg = qg_next, kg_next

    nc.compile()
    return nc


def make_in_maps(x, Wq, bq, Wk, bk, Wv, bv, Wp, bp, dt_mode="bf16"):
    """Shard full inputs into 8 per-core input maps."""
    import ml_dtypes
    npdt = ml_dtypes.bfloat16 if dt_mode == "bf16" else np.float32
    x = np.asarray(x, np.float32)
    scale = 1.0 / np.sqrt(D)
    tri = np.where(np.arange(P)[:, None] > np.arange(P)[None, :], 0.0, 1.0).astype(npdt)
    zeros_bp = np.zeros(OUTC, np.float32)
    in_maps = []
    for c in range(8):
        b, half = divmod(c, 2)
        sl = slice(half * CH, (half + 1) * CH)
        in_maps.append({
            "xT": np.ascontiguousarray(x[b].T).astype(npdt),
            "wqT": np.ascontiguousarray((np.asarray(Wq, np.float32)[sl] * scale).T).astype(npdt),
            "wkT": np.ascontiguousarray(np.asarray(Wk, np.float32)[sl].T).astype(npdt),
            "wvT": np.ascontiguousarray(np.asarray(Wv, np.float32)[sl].T).astype(npdt),
            "wpT": np.ascontiguousarray(np.asarray(Wp, np.float32)[:, sl].T).astype(npdt),
            "bq": (np.asarray(bq, np.float32)[sl] * scale).copy(),
            "bk": np.asarray(bk, np.float32)[sl].copy(),
            "bv": np.asarray(bv, np.float32)[sl].copy(),
            "bp": np.asarray(bp, np.float32).copy() if half == 0 else zeros_bp,
            "tri": tri,
        })
    return in_maps


def combine(results):
    """results: list of 8 dicts with 'out' [T, OUTC] partials -> [4, T, OUTC]."""
    return np.stack([results[2 * b]["out"] + results[2 * b + 1]["out"]
                     for b in range(4)]).astype(np.float32)


# ----------------------------------------------------------------------------
# Harness entry point: full inputs in, full output out.
# ----------------------------------------------------------------------------
_NC_CACHE = {}


def _get_nc(dt_mode):
    if dt_mode not in _NC_CACHE:
        _NC_CACHE[dt_mode] = build_nc(dt_mode)
    return _NC_CACHE[dt_mode]


def kernel(x, Wq, bq, Wk, bk, Wv, bv, Wp, bp):
    from concourse.bass_utils import run_bass_kernel_spmd
    dt_mode = "bf16"
    nc = _get_nc(dt_mode)
    in_maps = make_in_maps(x, Wq, bq, Wk, bk, Wv, bv, Wp, bp, dt_mode)
    res = run_bass_kernel_spmd(nc, in_maps, list(range(8)))
    return combine(res.results)
